# revision 7
# baseline (speedup 1.0000x reference)
"""Trainium2 Bass kernel for nn_DirectionalProcessor.

Math: the reference computes, for each pixel p=(h,w):
    out[p] = concat_d( shift_d(x)[p] @ Wd[d] ) @ Wc.T + bc
Because everything is linear, this collapses to an 8-tap 3x3 convolution
(zero center tap) with per-tap fused matrices:
    M_d = Wd[d] @ Wc[:, d*C:(d+1)*C].T          (C x C)
    out[p] = sum_d x[p - (dy_d, dx_d)] @ M_d + bc
This halves the FLOPs vs. the reference formulation. M_d is computed on
device (32 matmuls); the main loop is ~1056 accumulating matmuls per core.

Sharding: data-parallel over batch. 16 images / 8 cores = 2 images per core.
Weights are replicated to every core. No collectives.

Host does *layout only* (transpose/pad/zero-fill, no FLOPs):
  - grid  -> channel-major, zero-padded flat [2, 256, 4358] f32 per core
             (66x66 spatially padded image + 1 sentinel zero at each end,
             so every shifted tap window is a contiguous 1-D slice)
  - Wd    -> WdT  [8, e, c] (contraction dim e lands on partitions)
  - Wc    -> WcT  [8, e, o]
Device pipeline per core:
  - SWDGE cast-DMA fp32->fp16 for x and weights (PE fp16 matmul is 4x faster
    than fp32; rel. err ~1e-4, fp32 PSUM accumulation)
  - fold M_d on PE; bias broadcast [128,512] via rank-1 matmul (done once)
  - main loop: out tile = 128 consecutive *padded* positions x 256 channels;
    16 accumulating matmuls per tile (8 taps x 2 c-chunks); lhsT = contiguous
    128-wide window of the padded channel-major image, rhs = M_d chunk.
    Pad-column positions compute garbage that the host discards.
  - DVE adds bias while evacuating PSUM->SBUF (fp32), HWDGE DMA to a padded
    HBM output [64*66, 256] per image; host slices away the pad columns.
"""

import numpy as np

import concourse.bass as bass
import concourse.bacc as bacc
import concourse.mybir as mybir
import concourse.tile as tile
from concourse.bass_utils import run_bass_kernel_spmd

B, H, W, C = 16, 64, 64, 256
DIRECTIONS = [(0, -1), (1, -1), (1, 0), (1, 1), (0, 1), (-1, 1), (-1, 0), (-1, -1)]
N_CORES = 8
BPC = B // N_CORES  # images per core
HP = H + 2  # 66: padded spatial extent
XF = HP * HP + 2  # 4358: flat padded image + sentinel zero at each end
NQ = H * HP  # 4224: padded output positions per image (rows 1..64, all wp)
NT = (NQ + 127) // 128  # 33 output tiles per image
F16 = mybir.dt.float16
F32 = mybir.dt.float32

LAST_RESULTS = None  # test.py reads this for profiling info


def build_bass() -> bass.Bass:
    nc = bacc.Bacc(None)

    xp_d = nc.dram_tensor("xp", [BPC, C, XF], F32, kind="ExternalInput")
    wdt_d = nc.dram_tensor("wdt", [8, C, C], F32, kind="ExternalInput")  # [d, e, c]
    wct_d = nc.dram_tensor("wct", [8, C, C], F32, kind="ExternalInput")  # [d, e, o]
    b_d = nc.dram_tensor("bias", [1, 512], F32, kind="ExternalInput")
    out_d = nc.dram_tensor("out", [BPC * NQ, C], F32, kind="ExternalOutput")

    with tile.TileContext(nc) as tc:
        with (
            tc.tile_pool(name="const", bufs=1) as const,
            tc.tile_pool(name="psum", bufs=8, space="PSUM") as psum_pool,
            tc.tile_pool(name="osb", bufs=3) as osb_pool,
        ):
            # ---- weights: cast-load fp32 -> fp16 ----
            # layout [p=e%128, d, e_chunk, c|o] so e (contraction) is on partitions
            wdt16 = const.tile([128, 8, 2, C], F16, tag="wdt16")
            nc.gpsimd.dma_start(
                out=wdt16[:],
                in_=wdt_d[:].rearrange("d (ec p) c -> p d ec c", p=128),
            )
            wct16 = const.tile([128, 8, 2, C], F16, tag="wct16")
            nc.gpsimd.dma_start(
                out=wct16[:],
                in_=wct_d[:].rearrange("d (ec p) o -> p d ec o", p=128),
            )
            # single row: cols 0:128 = 1.0 (rank-1 lhsT), cols 256:512 = bc
            bias16 = const.tile([1, 512], F16, tag="bias16")
            nc.gpsimd.dma_start(out=bias16[:], in_=b_d[:])

            # ---- activations: cast-load fp32 -> fp16, channel-major padded ----
            xts = []  # [img][chunk] -> AP [128, XF]
            for img in range(BPC):
                per = []
                for ch in range(2):
                    t = const.tile([128, XF], F16, tag=f"xp_{img}_{ch}")
                    nc.gpsimd.dma_start(
                        out=t[:],
                        in_=xp_d[:][img, ch * 128 : (ch + 1) * 128],
                    )
                    per.append(t)
                xts.append(per)

            # ---- fold: M_d[c, o] = sum_e WdT[d][e, c] * WcT[d][e, o] ----
            # m16 layout [p=c%128, c_chunk, d, o]
            m16 = const.tile([128, 2, 8, C], F16, tag="m16")
            for d in range(8):
                mp = psum_pool.tile([128, 512], F32, tag="ps", name=f"mdps_{d}")
                for cc in range(2):
                    for ec in range(2):
                        nc.tensor.matmul(
                            mp[:, cc * 256 : (cc + 1) * 256],
                            lhsT=wdt16[:, d, ec, cc * 128 : (cc + 1) * 128],
                            rhs=wct16[:, d, ec, :],
                            start=(ec == 0),
                            stop=(ec == 1),
                        )
                for cc in range(2):
                    nc.vector.tensor_copy(
                        m16[:, cc, d, :], mp[:, cc * 256 : (cc + 1) * 256]
                    )

            # ---- bias broadcast to [128, 512] f32 via rank-1 matmul ----
            bp = psum_pool.tile([128, 512], F32, tag="ps", name="biasps")
            nc.tensor.matmul(bp[:, 0:256], lhsT=bias16[:, 0:128], rhs=bias16[:, 256:512])
            nc.tensor.matmul(bp[:, 256:512], lhsT=bias16[:, 0:128], rhs=bias16[:, 256:512])
            bias_sb = const.tile([128, 512], F32, tag="bias_sb")
            nc.vector.tensor_copy(bias_sb[:], bp[:])

            # ---- main conv loop ----
            # out tile j = padded positions q in [66 + 128j, 66 + 128j + 128);
            # tap d reads xpadbuf[1 + q + delta_d] -> contiguous slice start
            # 67 + 128j + delta_d. psum bank holds 2 out tiles.
            deltas = [-(dy * HP + dx) for (dx, dy) in DIRECTIONS]
            for img in range(BPC):
                x0, x1 = xts[img][0], xts[img][1]
                for g in range(5):  # tile groups: 8,8,8,8,1
                    gtiles = list(range(8 * g, min(8 * g + 8, NT)))
                    ow = len(gtiles) * 256
                    ot = osb_pool.tile(
                        [128, 2048], F32, tag="osb", name=f"ot{img}_{g}"
                    )
                    # 1-element touch: absorbs the slot-recycle wait so the
                    # bias-add TT below stays within the ISA sync-command limit
                    nc.vector.memset(ot[0:1, 0:1], 0.0)
                    for jp in range((len(gtiles) + 1) // 2):
                        pair = gtiles[jp * 2 : jp * 2 + 2]
                        pt = psum_pool.tile(
                            [128, 512], F32, tag="ps", name=f"ps{img}_{g}_{jp}"
                        )
                        for half, j in enumerate(pair):
                            for di in range(8):
                                s = 67 + 128 * j + deltas[di]
                                for ch, xt in enumerate((x0, x1)):
                                    nc.tensor.matmul(
                                        pt[:, half * 256 : (half + 1) * 256],
                                        lhsT=xt[:, s : s + 128],
                                        rhs=m16[:, ch, di, :],
                                        start=(di == 0 and ch == 0),
                                        stop=(di == 7 and ch == 1),
                                    )
                        pw = len(pair) * 256
                        nc.vector.tensor_add(
                            ot[:, jp * 512 : jp * 512 + pw],
                            pt[:, :pw],
                            bias_sb[:, :pw],
                        )
                    # store: out rows = img*NQ + 128*j + p, contiguous per tile
                    base = img * NQ + 128 * gtiles[0]
                    dst = out_d[:][base : base + 128 * len(gtiles), :].rearrange(
                        "(j p) o -> p j o", p=128
                    )
                    src = ot[:, :ow].rearrange("p (j o) -> p j o", o=256)
                    nc.sync.dma_start(out=dst, in_=src)

    nc.finalize()  # Bacc: run reg-alloc + sync-wait splitting before serialization
    return nc


def _host_prep(grid_embedding, Wd, Wc, bc):
    g = np.asarray(grid_embedding, dtype=np.float32)
    gpad = np.zeros((B, C, XF), np.float32)
    gview = gpad[:, :, 1 : 1 + HP * HP].reshape(B, C, HP, HP)
    gview[:, :, 1 : H + 1, 1 : W + 1] = g.transpose(0, 3, 1, 2)
    wdt = np.ascontiguousarray(
        np.asarray(Wd, np.float32).transpose(0, 2, 1)
    )  # [d, e, c]
    wct = np.ascontiguousarray(
        np.asarray(Wc, np.float32).reshape(C, 8, C).transpose(1, 2, 0)
    )  # [d, e, o]
    bias = np.zeros((1, 512), np.float32)
    bias[0, :128] = 1.0
    bias[0, 256:512] = np.asarray(bc, np.float32)
    return gpad, wdt, wct, bias


def _unpad_out(outpad_flat):
    # [NQ*images, 256] -> [images, H, W, C]: rows are (hp-1, wp) for padded
    # rows hp in 1..64 and all wp in 0..66; discard wp 0 and 65.
    n_img = outpad_flat.shape[0] // NQ
    o = outpad_flat.reshape(n_img, H, HP, C)
    return o[:, :, 1 : W + 1, :]


_NC_CACHE = {}


def kernel(grid_embedding, Wd, Wc, bc):
    global LAST_RESULTS
    gpad, wdt, wct, bias = _host_prep(grid_embedding, Wd, Wc, bc)

    if "nc" not in _NC_CACHE:
        _NC_CACHE["nc"] = build_bass()
    nc = _NC_CACHE["nc"]

    in_maps = [
        {
            "xp": np.ascontiguousarray(gpad[core * BPC : (core + 1) * BPC]),
            "wdt": wdt,
            "wct": wct,
            "bias": bias,
        }
        for core in range(N_CORES)
    ]
    res = run_bass_kernel_spmd(nc, in_maps, core_ids=list(range(N_CORES)))
    LAST_RESULTS = res
    out = np.concatenate([_unpad_out(r["out"]) for r in res.results], axis=0)
    return np.ascontiguousarray(out.reshape(B, H, W, C))


if __name__ == "__main__":
    rng = np.random.default_rng(0)
    inputs = {
        "grid_embedding": rng.standard_normal((B, H, W, C), dtype=np.float32),
        "Wd": (rng.standard_normal((8, C, C)) * 0.01).astype(np.float32),
        "Wc": (rng.standard_normal((C, 8 * C)) * 0.02).astype(np.float32),
        "bc": (rng.standard_normal(C) * 0.02).astype(np.float32),
    }
    out = kernel(**inputs)
    print("out", out.shape, out.dtype)


# revision 17
# speedup vs baseline: 31006.3238x; 31006.3238x over previous
"""Trainium2 Bass kernel for nn_DirectionalProcessor.

Math: the reference computes, for each pixel p=(h,w):
    out[p] = concat_d( shift_d(x)[p] @ Wd[d] ) @ Wc.T + bc
Because everything is linear, this collapses to an 8-tap 3x3 convolution
(zero center tap) with per-tap fused matrices:
    M_d = Wd[d] @ Wc[:, d*C:(d+1)*C].T          (C x C)
    out[p] = sum_d x[p - (dy_d, dx_d)] @ M_d + bc
This halves the FLOPs vs. the reference formulation. M_d is computed on
device (32 matmuls); the main loop is ~1056 accumulating matmuls per core.

Sharding: data-parallel over batch. 16 images / 8 cores = 2 images per core.
Weights are replicated to every core. No collectives.

Host does *layout only* (transpose/pad/zero-fill, no FLOPs):
  - grid  -> channel-major, zero-padded flat [2, 256, 4358] f32 per core
             (66x66 spatially padded image + 1 sentinel zero at each end,
             so every shifted tap window is a contiguous 1-D slice)
  - Wd    -> WdT  [8, e, c] (contraction dim e lands on partitions)
  - Wc    -> WcT  [8, e, o]
Device pipeline per core:
  - SWDGE cast-DMA fp32->fp16 for x and weights (PE fp16 matmul is 4x faster
    than fp32; rel. err ~1e-4, fp32 PSUM accumulation)
  - fold M_d on PE; bias broadcast [128,512] via rank-1 matmul (done once)
  - main loop: out tile = 128 consecutive *padded* positions x 256 channels;
    16 accumulating matmuls per tile (8 taps x 2 c-chunks); lhsT = contiguous
    128-wide window of the padded channel-major image, rhs = M_d chunk.
    Pad-column positions compute garbage that the host discards.
  - DVE adds bias while evacuating PSUM->SBUF (fp32), HWDGE DMA to a padded
    HBM output [64*66, 256] per image; host slices away the pad columns.
"""

import numpy as np

import concourse.bass as bass
import concourse.bacc as bacc
import concourse.mybir as mybir
import concourse.tile as tile
from concourse.bass_utils import run_bass_kernel_spmd

B, H, W, C = 16, 64, 64, 256
DIRECTIONS = [(0, -1), (1, -1), (1, 0), (1, 1), (0, 1), (-1, 1), (-1, 0), (-1, -1)]
N_CORES = 8
BPC = B // N_CORES  # images per core
HP = H + 2  # 66: padded spatial extent
XF = HP * HP + 2  # 4358: flat padded image + sentinel zero at each end
NQ = H * HP  # 4224: padded output positions per image (rows 1..64, all wp)
NT = (NQ + 127) // 128  # 33 output tiles per image
F16 = mybir.dt.float16
F32 = mybir.dt.float32
F32R = mybir.dt.float32r  # fp32 storage, single-pass PE mode (full rate at N>=256)

LAST_RESULTS = None  # test.py reads this for profiling info


def build_bass() -> bass.Bass:
    nc = bacc.Bacc(None)

    xp_d = nc.dram_tensor("xp", [BPC, C, XF], F32, kind="ExternalInput")
    wdt_d = nc.dram_tensor("wdt", [8, C, C], F32R, kind="ExternalInput")  # [d, e, c]
    wct_d = nc.dram_tensor("wct", [8, C, C], F32R, kind="ExternalInput")  # [d, e, o]
    b_d = nc.dram_tensor("bias", [1, 512], F32, kind="ExternalInput")
    out_d = nc.dram_tensor("out", [BPC * NQ, C], F32, kind="ExternalOutput")

    with tile.TileContext(nc) as tc:
        with (
            tc.tile_pool(name="const", bufs=1) as const,
            tc.tile_pool(name="psum", bufs=7, space="PSUM") as psum_pool,
            tc.tile_pool(name="warmps", bufs=1, space="PSUM") as warm_pool,
            tc.tile_pool(name="osb", bufs=3) as osb_pool,
        ):
            # ---- PE pre-warm: dummy matmuls fill the weight-DMA window so the
            # HAM clock gate is at 2.4 GHz when real work arrives ----
            warm16 = const.tile([128, 512], F16, tag="warm16")
            nc.vector.memset(warm16[:], 0.0)
            wps = warm_pool.tile([128, 512], F32, tag="warm")
            for _ in range(18):
                nc.tensor.matmul(wps[:], lhsT=warm16[:, 0:128], rhs=warm16[:])
            # ---- weights: HWDGE fp32r loads, split by direction halves so the
            # fold can start as soon as the first half lands ----
            # layout [p=e%128, d, e_chunk, c|o] so e (contraction) is on partitions
            # single SWDGE FIFO carries every input DMA in priority order:
            # weight halves -> bias -> img0 strips -> img1 strips
            wdt32 = const.tile([128, 8, 2, C], F32R, tag="wdt32")
            wct32 = const.tile([128, 8, 2, C], F32R, tag="wct32")
            for lo in (0, 2, 4, 6):
                nc.gpsimd.dma_start(
                    out=wdt32[:, lo : lo + 2],
                    in_=wdt_d[:][lo : lo + 2].rearrange("d (ec p) c -> p d ec c", p=128),
                )
                nc.gpsimd.dma_start(
                    out=wct32[:, lo : lo + 2],
                    in_=wct_d[:][lo : lo + 2].rearrange("d (ec p) o -> p d ec o", p=128),
                )
            # single row: cols 0:128 = 1.0 (rank-1 lhsT), cols 256:512 = bc
            bias16 = const.tile([1, 512], F16, tag="bias16")
            nc.gpsimd.dma_start(out=bias16[:], in_=b_d[:])

            # ---- activations: cast-load fp32 -> fp16, channel-major padded,
            # strip-mined so early conv tiles unblock before the full image lands
            NSTRIP = 2
            sbounds = [XF * s // NSTRIP for s in range(NSTRIP + 1)]
            xts = []  # [img][chunk] -> AP [128, XF]
            for img in range(BPC):
                per = []
                for ch in range(2):
                    t = const.tile([128, XF], F16, tag=f"xp_{img}_{ch}")
                    per.append(t)
                xts.append(per)
            for img in range(BPC):
                for s in range(NSTRIP):
                    a, b = sbounds[s], sbounds[s + 1]
                    for ch in range(2):
                        nc.gpsimd.dma_start(
                            out=xts[img][ch][:, a:b],
                            in_=xp_d[:][img, ch * 128 : (ch + 1) * 128, a:b],
                        )

            # ---- fold: M_d[c, o] = sum_e WdT[d][e, c] * WcT[d][e, o] ----
            # m16 layout [p=c%128, c_chunk, d, o]
            m16 = const.tile([128, 2, 8, C], F16, tag="m16")
            for d in range(8):
                mp = psum_pool.tile([128, 512], F32, tag="ps", name=f"mdps_{d}")
                for cc in range(2):
                    for ec in range(2):
                        nc.tensor.matmul(
                            mp[:, cc * 256 : (cc + 1) * 256],
                            lhsT=wdt32[:, d, ec, cc * 128 : (cc + 1) * 128],
                            rhs=wct32[:, d, ec, :],
                            start=(ec == 0),
                            stop=(ec == 1),
                        )
                nc.vector.tensor_copy(m16[:, :, d, :], mp[:])

            # ---- bias broadcast to [128, 512] f32 via rank-1 matmul ----
            bp = psum_pool.tile([128, 512], F32, tag="ps", name="biasps")
            nc.tensor.matmul(bp[:, 0:256], lhsT=bias16[:, 0:128], rhs=bias16[:, 256:512])
            nc.tensor.matmul(bp[:, 256:512], lhsT=bias16[:, 0:128], rhs=bias16[:, 256:512])
            bias_sb = const.tile([128, 512], F32, tag="bias_sb")
            nc.vector.tensor_copy(bias_sb[:], bp[:])

            # ---- main conv loop ----
            # out tile j = padded positions q in [66 + 128j, 66 + 128j + 128);
            # tap d reads xpadbuf[1 + q + delta_d] -> contiguous slice start
            # 67 + 128j + delta_d. psum bank holds 2 out tiles.
            deltas = [-(dy * HP + dx) for (dx, dy) in DIRECTIONS]
            for img in range(BPC):
                x0, x1 = xts[img][0], xts[img][1]
                for g in range(5):  # tile groups: 8,8,8,8,1
                    gtiles = list(range(8 * g, min(8 * g + 8, NT)))
                    ow = len(gtiles) * 256
                    ot = osb_pool.tile(
                        [128, 2048], F32, tag="osb", name=f"ot{img}_{g}"
                    )
                    # 1-element touch: absorbs the slot-recycle wait so the
                    # bias-add TT below stays within the ISA sync-command limit
                    nc.vector.memset(ot[0:1, 0:1], 0.0)
                    for jp in range((len(gtiles) + 1) // 2):
                        pair = gtiles[jp * 2 : jp * 2 + 2]
                        pt = psum_pool.tile(
                            [128, 512], F32, tag="ps", name=f"ps{img}_{g}_{jp}"
                        )
                        for half, j in enumerate(pair):
                            for di in range(8):
                                s = 67 + 128 * j + deltas[di]
                                for ch, xt in enumerate((x0, x1)):
                                    nc.tensor.matmul(
                                        pt[:, half * 256 : (half + 1) * 256],
                                        lhsT=xt[:, s : s + 128],
                                        rhs=m16[:, ch, di, :],
                                        start=(di == 0 and ch == 0),
                                        stop=(di == 7 and ch == 1),
                                    )
                        pw = len(pair) * 256
                        nc.vector.tensor_add(
                            ot[:, jp * 512 : jp * 512 + pw],
                            pt[:, :pw],
                            bias_sb[:, :pw],
                        )
                    # store: out rows = img*NQ + 128*j + p, contiguous per tile
                    base = img * NQ + 128 * gtiles[0]
                    dst = out_d[:][base : base + 128 * len(gtiles), :].rearrange(
                        "(j p) o -> p j o", p=128
                    )
                    src = ot[:, :ow].rearrange("p (j o) -> p j o", o=256)
                    nc.sync.dma_start(out=dst, in_=src)

    nc.finalize()  # Bacc: run reg-alloc + sync-wait splitting before serialization
    return nc


def _host_prep(grid_embedding, Wd, Wc, bc):
    g = np.asarray(grid_embedding, dtype=np.float32)
    gpad = np.zeros((B, C, XF), np.float32)
    gview = gpad[:, :, 1 : 1 + HP * HP].reshape(B, C, HP, HP)
    gview[:, :, 1 : H + 1, 1 : W + 1] = g.transpose(0, 3, 1, 2)
    wdt = np.ascontiguousarray(
        np.asarray(Wd, np.float32).transpose(0, 2, 1)
    )  # [d, e, c]
    wct = np.ascontiguousarray(
        np.asarray(Wc, np.float32).reshape(C, 8, C).transpose(1, 2, 0)
    )  # [d, e, o]
    bias = np.zeros((1, 512), np.float32)
    bias[0, :128] = 1.0
    bias[0, 256:512] = np.asarray(bc, np.float32)
    return gpad, wdt, wct, bias


def _unpad_out(outpad_flat):
    # [NQ*images, 256] -> [images, H, W, C]: rows are (hp-1, wp) for padded
    # rows hp in 1..64 and all wp in 0..66; discard wp 0 and 65.
    n_img = outpad_flat.shape[0] // NQ
    o = outpad_flat.reshape(n_img, H, HP, C)
    return o[:, :, 1 : W + 1, :]


_NC_CACHE = {}


def kernel(grid_embedding, Wd, Wc, bc):
    global LAST_RESULTS
    gpad, wdt, wct, bias = _host_prep(grid_embedding, Wd, Wc, bc)

    if "nc" not in _NC_CACHE:
        _NC_CACHE["nc"] = build_bass()
    nc = _NC_CACHE["nc"]

    in_maps = [
        {
            "xp": np.ascontiguousarray(gpad[core * BPC : (core + 1) * BPC]),
            "wdt": wdt,
            "wct": wct,
            "bias": bias,
        }
        for core in range(N_CORES)
    ]
    res = run_bass_kernel_spmd(nc, in_maps, core_ids=list(range(N_CORES)))
    LAST_RESULTS = res
    out = np.concatenate([_unpad_out(r["out"]) for r in res.results], axis=0)
    return np.ascontiguousarray(out.reshape(B, H, W, C))


if __name__ == "__main__":
    rng = np.random.default_rng(0)
    inputs = {
        "grid_embedding": rng.standard_normal((B, H, W, C), dtype=np.float32),
        "Wd": (rng.standard_normal((8, C, C)) * 0.01).astype(np.float32),
        "Wc": (rng.standard_normal((C, 8 * C)) * 0.02).astype(np.float32),
        "bc": (rng.standard_normal(C) * 0.02).astype(np.float32),
    }
    out = kernel(**inputs)
    print("out", out.shape, out.dtype)


# revision 21
# speedup vs baseline: 31092.5506x; 1.0028x over previous
"""Trainium2 Bass kernel for nn_DirectionalProcessor.

Math: the reference computes, for each pixel p=(h,w):
    out[p] = concat_d( shift_d(x)[p] @ Wd[d] ) @ Wc.T + bc
Because everything is linear, this collapses to an 8-tap 3x3 convolution
(zero center tap) with per-tap fused matrices:
    M_d = Wd[d] @ Wc[:, d*C:(d+1)*C].T          (C x C)
    out[p] = sum_d x[p - (dy_d, dx_d)] @ M_d + bc
This halves the FLOPs vs. the reference formulation. M_d is computed on
device (32 matmuls); the main loop is ~1056 accumulating matmuls per core.

Sharding: data-parallel over batch. 16 images / 8 cores = 2 images per core.
Weights are replicated to every core. No collectives.

Host does *layout only* (transpose/pad/zero-fill, no FLOPs):
  - grid  -> channel-major, zero-padded flat [2, 256, 4358] f32 per core
             (66x66 spatially padded image + 1 sentinel zero at each end,
             so every shifted tap window is a contiguous 1-D slice)
  - Wd    -> WdT  [8, e, c] (contraction dim e lands on partitions)
  - Wc    -> WcT  [8, e, o]
Device pipeline per core:
  - SWDGE cast-DMA fp32->fp16 for x and weights (PE fp16 matmul is 4x faster
    than fp32; rel. err ~1e-4, fp32 PSUM accumulation)
  - fold M_d on PE; bias broadcast [128,512] via rank-1 matmul (done once)
  - main loop: out tile = 128 consecutive *padded* positions x 256 channels;
    16 accumulating matmuls per tile (8 taps x 2 c-chunks); lhsT = contiguous
    128-wide window of the padded channel-major image, rhs = M_d chunk.
    Pad-column positions compute garbage that the host discards.
  - DVE adds bias while evacuating PSUM->SBUF (fp32), HWDGE DMA to a padded
    HBM output [64*66, 256] per image; host slices away the pad columns.
"""

import numpy as np

import concourse.bass as bass
import concourse.bacc as bacc
import concourse.mybir as mybir
import concourse.tile as tile
from concourse.bass_utils import run_bass_kernel_spmd

B, H, W, C = 16, 64, 64, 256
DIRECTIONS = [(0, -1), (1, -1), (1, 0), (1, 1), (0, 1), (-1, 1), (-1, 0), (-1, -1)]
N_CORES = 8
BPC = B // N_CORES  # images per core
HP = H + 2  # 66: padded spatial extent
XF = HP * HP + 2  # 4358: flat padded image + sentinel zero at each end
NQ = H * HP  # 4224: padded output positions per image (rows 1..64, all wp)
NT = (NQ + 127) // 128  # 33 output tiles per image
F16 = mybir.dt.float16
F32 = mybir.dt.float32
F32R = mybir.dt.float32r  # fp32 storage, single-pass PE mode (full rate at N>=256)

LAST_RESULTS = None  # test.py reads this for profiling info


def build_bass() -> bass.Bass:
    nc = bacc.Bacc(None)

    xp_d = nc.dram_tensor("xp", [BPC, C, XF], F32, kind="ExternalInput")
    # weights arrive host-permuted to the exact SBUF layout [p=e%128, d, ec, c|o]
    # so the loads are contiguous line-rate DMAs
    wdt_d = nc.dram_tensor("wdt", [128, 8, 2, C], F32R, kind="ExternalInput")
    wct_d = nc.dram_tensor("wct", [128, 8, 2, C], F32R, kind="ExternalInput")
    b_d = nc.dram_tensor("bias", [1, 512], F32, kind="ExternalInput")
    out_d = nc.dram_tensor("out", [BPC * NQ, C], F32, kind="ExternalOutput")

    with tile.TileContext(nc) as tc:
        with (
            tc.tile_pool(name="const", bufs=1) as const,
            tc.tile_pool(name="psum", bufs=7, space="PSUM") as psum_pool,
            tc.tile_pool(name="warmps", bufs=1, space="PSUM") as warm_pool,
            tc.tile_pool(name="osb", bufs=3) as osb_pool,
        ):
            # ---- PE pre-warm: dummy matmuls fill the weight-DMA window so the
            # HAM clock gate is at 2.4 GHz when real work arrives ----
            warm16 = const.tile([128, 512], F16, tag="warm16")
            nc.vector.memset(warm16[:], 0.0)
            wps = warm_pool.tile([128, 512], F32, tag="warm")
            for _ in range(18):
                nc.tensor.matmul(wps[:], lhsT=warm16[:, 0:128], rhs=warm16[:])
            # ---- weights: HWDGE fp32r loads, split by direction halves so the
            # fold can start as soon as the first half lands ----
            # layout [p=e%128, d, e_chunk, c|o] so e (contraction) is on partitions
            # single SWDGE FIFO carries every input DMA in priority order:
            # weight halves -> bias -> img0 strips -> img1 strips
            wdt32 = const.tile([128, 8, 2, C], F32R, tag="wdt32")
            wct32 = const.tile([128, 8, 2, C], F32R, tag="wct32")
            for lo in (0, 2, 4, 6):
                nc.gpsimd.dma_start(
                    out=wdt32[:, lo : lo + 2], in_=wdt_d[:][:, lo : lo + 2]
                )
                nc.gpsimd.dma_start(
                    out=wct32[:, lo : lo + 2], in_=wct_d[:][:, lo : lo + 2]
                )
            # single row: cols 0:128 = 1.0 (rank-1 lhsT), cols 256:512 = bc
            bias16 = const.tile([1, 512], F16, tag="bias16")
            nc.gpsimd.dma_start(out=bias16[:], in_=b_d[:])

            # ---- activations: cast-load fp32 -> fp16, channel-major padded.
            # The SWDGE ring drains in issue order at ~350 GB/s, so the layout
            # of this DMA chain IS the startup schedule: a small first strip
            # (1024 cols) of image 0 lands right as the weight fold finishes,
            # unblocking the first conv tiles; the rest streams in behind.
            # Total gpsimd DMAs kept at 15 so 8-sem-lane reuse waits are
            # always on long-completed transfers.
            S0 = 1024
            xts = []  # [img][chunk] -> AP [128, XF]
            for img in range(BPC):
                per = []
                for ch in range(2):
                    t = const.tile([128, XF], F16, tag=f"xp_{img}_{ch}")
                    per.append(t)
                xts.append(per)
            for ch in range(2):  # img0 small head strips
                nc.gpsimd.dma_start(
                    out=xts[0][ch][:, 0:S0],
                    in_=xp_d[:][0, ch * 128 : (ch + 1) * 128, 0:S0],
                )
            for ch in range(2):  # img0 remainder
                nc.gpsimd.dma_start(
                    out=xts[0][ch][:, S0:XF],
                    in_=xp_d[:][0, ch * 128 : (ch + 1) * 128, S0:XF],
                )
            for ch in range(2):  # img1 whole
                nc.gpsimd.dma_start(
                    out=xts[1][ch][:],
                    in_=xp_d[:][1, ch * 128 : (ch + 1) * 128],
                )

            # ---- fold: M_d[c, o] = sum_e WdT[d][e, c] * WcT[d][e, o] ----
            # m16 layout [p=c%128, c_chunk, d, o]
            m16 = const.tile([128, 2, 8, C], F16, tag="m16")
            for d in range(8):
                mp = psum_pool.tile([128, 512], F32, tag="ps", name=f"mdps_{d}")
                for cc in range(2):
                    for ec in range(2):
                        nc.tensor.matmul(
                            mp[:, cc * 256 : (cc + 1) * 256],
                            lhsT=wdt32[:, d, ec, cc * 128 : (cc + 1) * 128],
                            rhs=wct32[:, d, ec, :],
                            start=(ec == 0),
                            stop=(ec == 1),
                        )
                nc.vector.tensor_copy(m16[:, :, d, :], mp[:])

            # ---- bias broadcast to [128, 512] f32 via rank-1 matmul ----
            bp = psum_pool.tile([128, 512], F32, tag="ps", name="biasps")
            nc.tensor.matmul(bp[:, 0:256], lhsT=bias16[:, 0:128], rhs=bias16[:, 256:512])
            nc.tensor.matmul(bp[:, 256:512], lhsT=bias16[:, 0:128], rhs=bias16[:, 256:512])
            bias_sb = const.tile([128, 512], F32, tag="bias_sb")
            nc.vector.tensor_copy(bias_sb[:], bp[:])

            # ---- main conv loop ----
            # out tile j = padded positions q in [66 + 128j, 66 + 128j + 128);
            # tap d reads xpadbuf[1 + q + delta_d] -> contiguous slice start
            # 67 + 128j + delta_d. psum bank holds 2 out tiles.
            deltas = [-(dy * HP + dx) for (dx, dy) in DIRECTIONS]
            for img in range(BPC):
                x0, x1 = xts[img][0], xts[img][1]
                for g in range(5):  # tile groups: 8,8,8,8,1
                    gtiles = list(range(8 * g, min(8 * g + 8, NT)))
                    ow = len(gtiles) * 256
                    ot = osb_pool.tile(
                        [128, 2048], F32, tag="osb", name=f"ot{img}_{g}"
                    )
                    # 1-element touch: absorbs the slot-recycle wait so the
                    # bias-add TT below stays within the ISA sync-command limit
                    nc.vector.memset(ot[0:1, 0:1], 0.0)
                    for jp in range((len(gtiles) + 1) // 2):
                        pair = gtiles[jp * 2 : jp * 2 + 2]
                        pt = psum_pool.tile(
                            [128, 512], F32, tag="ps", name=f"ps{img}_{g}_{jp}"
                        )
                        for half, j in enumerate(pair):
                            for di in range(8):
                                s = 67 + 128 * j + deltas[di]
                                for ch, xt in enumerate((x0, x1)):
                                    nc.tensor.matmul(
                                        pt[:, half * 256 : (half + 1) * 256],
                                        lhsT=xt[:, s : s + 128],
                                        rhs=m16[:, ch, di, :],
                                        start=(di == 0 and ch == 0),
                                        stop=(di == 7 and ch == 1),
                                    )
                        pw = len(pair) * 256
                        nc.vector.tensor_add(
                            ot[:, jp * 512 : jp * 512 + pw],
                            pt[:, :pw],
                            bias_sb[:, :pw],
                        )
                    # store: out rows = img*NQ + 128*j + p, contiguous per tile
                    base = img * NQ + 128 * gtiles[0]
                    dst = out_d[:][base : base + 128 * len(gtiles), :].rearrange(
                        "(j p) o -> p j o", p=128
                    )
                    src = ot[:, :ow].rearrange("p (j o) -> p j o", o=256)
                    nc.sync.dma_start(out=dst, in_=src)

    nc.finalize()  # Bacc: run reg-alloc + sync-wait splitting before serialization
    return nc


def _host_prep(grid_embedding, Wd, Wc, bc):
    g = np.asarray(grid_embedding, dtype=np.float32)
    gpad = np.zeros((B, C, XF), np.float32)
    gview = gpad[:, :, 1 : 1 + HP * HP].reshape(B, C, HP, HP)
    gview[:, :, 1 : H + 1, 1 : W + 1] = g.transpose(0, 3, 1, 2)
    # [d, e, c] / [d, e, o], then permuted to the SBUF layout [p=e%128, d, ec, c|o]
    wdt_dec = np.asarray(Wd, np.float32).transpose(0, 2, 1)
    wct_dec = np.asarray(Wc, np.float32).reshape(C, 8, C).transpose(1, 2, 0)
    wdt = np.ascontiguousarray(
        wdt_dec.reshape(8, 2, 128, C).transpose(2, 0, 1, 3)
    )  # [128, 8, 2, C]
    wct = np.ascontiguousarray(
        wct_dec.reshape(8, 2, 128, C).transpose(2, 0, 1, 3)
    )  # [128, 8, 2, C]
    bias = np.zeros((1, 512), np.float32)
    bias[0, :128] = 1.0
    bias[0, 256:512] = np.asarray(bc, np.float32)
    return gpad, wdt, wct, bias


def _unpad_out(outpad_flat):
    # [NQ*images, 256] -> [images, H, W, C]: rows are (hp-1, wp) for padded
    # rows hp in 1..64 and all wp in 0..66; discard wp 0 and 65.
    n_img = outpad_flat.shape[0] // NQ
    o = outpad_flat.reshape(n_img, H, HP, C)
    return o[:, :, 1 : W + 1, :]


_NC_CACHE = {}


def kernel(grid_embedding, Wd, Wc, bc):
    global LAST_RESULTS
    gpad, wdt, wct, bias = _host_prep(grid_embedding, Wd, Wc, bc)

    if "nc" not in _NC_CACHE:
        _NC_CACHE["nc"] = build_bass()
    nc = _NC_CACHE["nc"]

    in_maps = [
        {
            "xp": np.ascontiguousarray(gpad[core * BPC : (core + 1) * BPC]),
            "wdt": wdt,
            "wct": wct,
            "bias": bias,
        }
        for core in range(N_CORES)
    ]
    res = run_bass_kernel_spmd(nc, in_maps, core_ids=list(range(N_CORES)))
    LAST_RESULTS = res
    out = np.concatenate([_unpad_out(r["out"]) for r in res.results], axis=0)
    return np.ascontiguousarray(out.reshape(B, H, W, C))


if __name__ == "__main__":
    rng = np.random.default_rng(0)
    inputs = {
        "grid_embedding": rng.standard_normal((B, H, W, C), dtype=np.float32),
        "Wd": (rng.standard_normal((8, C, C)) * 0.01).astype(np.float32),
        "Wc": (rng.standard_normal((C, 8 * C)) * 0.02).astype(np.float32),
        "bc": (rng.standard_normal(C) * 0.02).astype(np.float32),
    }
    out = kernel(**inputs)
    print("out", out.shape, out.dtype)


# revision 22
# speedup vs baseline: 32942.4382x; 1.0595x over previous
"""Trainium2 Bass kernel for nn_DirectionalProcessor.

Math: the reference computes, for each pixel p=(h,w):
    out[p] = concat_d( shift_d(x)[p] @ Wd[d] ) @ Wc.T + bc
Because everything is linear, this collapses to an 8-tap 3x3 convolution
(zero center tap) with per-tap fused matrices:
    M_d = Wd[d] @ Wc[:, d*C:(d+1)*C].T          (C x C)
    out[p] = sum_d x[p - (dy_d, dx_d)] @ M_d + bc
This halves the FLOPs vs. the reference formulation. M_d is computed on
device (32 matmuls); the main loop is ~1056 accumulating matmuls per core.

Sharding: data-parallel over batch. 16 images / 8 cores = 2 images per core.
Weights are replicated to every core. No collectives.

Host does *layout only* (transpose/pad/zero-fill, no FLOPs):
  - grid  -> channel-major, zero-padded flat [2, 256, 4358] f32 per core
             (66x66 spatially padded image + 1 sentinel zero at each end,
             so every shifted tap window is a contiguous 1-D slice)
  - Wd    -> WdT  [8, e, c] (contraction dim e lands on partitions)
  - Wc    -> WcT  [8, e, o]
Device pipeline per core:
  - SWDGE cast-DMA fp32->fp16 for x and weights (PE fp16 matmul is 4x faster
    than fp32; rel. err ~1e-4, fp32 PSUM accumulation)
  - fold M_d on PE; bias broadcast [128,512] via rank-1 matmul (done once)
  - main loop: out tile = 128 consecutive *padded* positions x 256 channels;
    16 accumulating matmuls per tile (8 taps x 2 c-chunks); lhsT = contiguous
    128-wide window of the padded channel-major image, rhs = M_d chunk.
    Pad-column positions compute garbage that the host discards.
  - DVE adds bias while evacuating PSUM->SBUF (fp32), HWDGE DMA to a padded
    HBM output [64*66, 256] per image; host slices away the pad columns.
"""

import numpy as np

import concourse.bass as bass
import concourse.bacc as bacc
import concourse.mybir as mybir
import concourse.tile as tile
from concourse.bass_utils import run_bass_kernel_spmd

B, H, W, C = 16, 64, 64, 256
DIRECTIONS = [(0, -1), (1, -1), (1, 0), (1, 1), (0, 1), (-1, 1), (-1, 0), (-1, -1)]
N_CORES = 8
BPC = B // N_CORES  # images per core
HP = H + 2  # 66: padded spatial extent
XF = HP * HP + 2  # 4358: flat padded image + sentinel zero at each end
NQ = H * HP  # 4224: padded output positions per image (rows 1..64, all wp)
NT = (NQ + 127) // 128  # 33 output tiles per image
F16 = mybir.dt.float16
F32 = mybir.dt.float32
F32R = mybir.dt.float32r  # fp32 storage, single-pass PE mode (full rate at N>=256)

LAST_RESULTS = None  # test.py reads this for profiling info


def build_bass() -> bass.Bass:
    nc = bacc.Bacc(None)

    xp_d = nc.dram_tensor("xp", [BPC, C, XF], F32, kind="ExternalInput")
    # weights arrive host-permuted to the exact SBUF layout [p=e%128, d, ec, c|o]
    # so the loads are contiguous line-rate DMAs
    wdt_d = nc.dram_tensor("wdt", [128, 8, 2, C], F16, kind="ExternalInput")
    wct_d = nc.dram_tensor("wct", [128, 8, 2, C], F16, kind="ExternalInput")
    b_d = nc.dram_tensor("bias", [1, 512], F32, kind="ExternalInput")
    out_d = nc.dram_tensor("out", [BPC * NQ, C], F32, kind="ExternalOutput")

    with tile.TileContext(nc) as tc:
        with (
            tc.tile_pool(name="const", bufs=1) as const,
            tc.tile_pool(name="psum", bufs=7, space="PSUM") as psum_pool,
            tc.tile_pool(name="warmps", bufs=1, space="PSUM") as warm_pool,
            tc.tile_pool(name="osb", bufs=3) as osb_pool,
        ):
            # ---- PE pre-warm: dummy matmuls fill the weight-DMA window so the
            # HAM clock gate is at 2.4 GHz when real work arrives ----
            warm16 = const.tile([128, 512], F16, tag="warm16")
            nc.vector.memset(warm16[:], 0.0)
            wps = warm_pool.tile([128, 512], F32, tag="warm")
            for _ in range(18):
                nc.tensor.matmul(wps[:], lhsT=warm16[:, 0:128], rhs=warm16[:])
            # ---- weights: HWDGE fp32r loads, split by direction halves so the
            # fold can start as soon as the first half lands ----
            # layout [p=e%128, d, e_chunk, c|o] so e (contraction) is on partitions
            # single SWDGE FIFO carries every input DMA in priority order:
            # weight halves -> bias -> img0 strips -> img1 strips
            wdt32 = const.tile([128, 8, 2, C], F16, tag="wdt32")
            wct32 = const.tile([128, 8, 2, C], F16, tag="wct32")
            for lo in (0, 2, 4, 6):
                nc.gpsimd.dma_start(
                    out=wdt32[:, lo : lo + 2], in_=wdt_d[:][:, lo : lo + 2]
                )
                nc.gpsimd.dma_start(
                    out=wct32[:, lo : lo + 2], in_=wct_d[:][:, lo : lo + 2]
                )
            # single row: cols 0:128 = 1.0 (rank-1 lhsT), cols 256:512 = bc
            bias16 = const.tile([1, 512], F16, tag="bias16")
            nc.gpsimd.dma_start(out=bias16[:], in_=b_d[:])

            # ---- activations: cast-load fp32 -> fp16, channel-major padded.
            # The SWDGE ring drains in issue order at ~350 GB/s, so the layout
            # of this DMA chain IS the startup schedule: a small first strip
            # (1024 cols) of image 0 lands right as the weight fold finishes,
            # unblocking the first conv tiles; the rest streams in behind.
            # Total gpsimd DMAs kept at 15 so 8-sem-lane reuse waits are
            # always on long-completed transfers.
            S0 = 1024
            xts = []  # [img][chunk] -> AP [128, XF]
            for img in range(BPC):
                per = []
                for ch in range(2):
                    t = const.tile([128, XF], F16, tag=f"xp_{img}_{ch}")
                    per.append(t)
                xts.append(per)
            for ch in range(2):  # img0 small head strips
                nc.gpsimd.dma_start(
                    out=xts[0][ch][:, 0:S0],
                    in_=xp_d[:][0, ch * 128 : (ch + 1) * 128, 0:S0],
                )
            for ch in range(2):  # img0 remainder
                nc.gpsimd.dma_start(
                    out=xts[0][ch][:, S0:XF],
                    in_=xp_d[:][0, ch * 128 : (ch + 1) * 128, S0:XF],
                )
            for ch in range(2):  # img1 whole
                nc.gpsimd.dma_start(
                    out=xts[1][ch][:],
                    in_=xp_d[:][1, ch * 128 : (ch + 1) * 128],
                )

            # ---- fold: M_d[c, o] = sum_e WdT[d][e, c] * WcT[d][e, o] ----
            # m16 layout [p=c%128, c_chunk, d, o]
            m16 = const.tile([128, 2, 8, C], F16, tag="m16")
            for d in range(8):
                mp = psum_pool.tile([128, 512], F32, tag="ps", name=f"mdps_{d}")
                for cc in range(2):
                    for ec in range(2):
                        nc.tensor.matmul(
                            mp[:, cc * 256 : (cc + 1) * 256],
                            lhsT=wdt32[:, d, ec, cc * 128 : (cc + 1) * 128],
                            rhs=wct32[:, d, ec, :],
                            start=(ec == 0),
                            stop=(ec == 1),
                        )
                nc.vector.tensor_copy(m16[:, :, d, :], mp[:])

            # ---- bias broadcast to [128, 512] f32 via rank-1 matmul ----
            bp = psum_pool.tile([128, 512], F32, tag="ps", name="biasps")
            nc.tensor.matmul(bp[:, 0:256], lhsT=bias16[:, 0:128], rhs=bias16[:, 256:512])
            nc.tensor.matmul(bp[:, 256:512], lhsT=bias16[:, 0:128], rhs=bias16[:, 256:512])
            bias_sb = const.tile([128, 512], F32, tag="bias_sb")
            nc.vector.tensor_copy(bias_sb[:], bp[:])

            # ---- main conv loop ----
            # out tile j = padded positions q in [66 + 128j, 66 + 128j + 128);
            # tap d reads xpadbuf[1 + q + delta_d] -> contiguous slice start
            # 67 + 128j + delta_d. psum bank holds 2 out tiles.
            deltas = [-(dy * HP + dx) for (dx, dy) in DIRECTIONS]
            for img in range(BPC):
                x0, x1 = xts[img][0], xts[img][1]
                for g in range(5):  # tile groups: 8,8,8,8,1
                    gtiles = list(range(8 * g, min(8 * g + 8, NT)))
                    ow = len(gtiles) * 256
                    ot = osb_pool.tile(
                        [128, 2048], F32, tag="osb", name=f"ot{img}_{g}"
                    )
                    # 1-element touch: absorbs the slot-recycle wait so the
                    # bias-add TT below stays within the ISA sync-command limit
                    nc.vector.memset(ot[0:1, 0:1], 0.0)
                    for jp in range((len(gtiles) + 1) // 2):
                        pair = gtiles[jp * 2 : jp * 2 + 2]
                        pt = psum_pool.tile(
                            [128, 512], F32, tag="ps", name=f"ps{img}_{g}_{jp}"
                        )
                        for half, j in enumerate(pair):
                            for di in range(8):
                                s = 67 + 128 * j + deltas[di]
                                for ch, xt in enumerate((x0, x1)):
                                    nc.tensor.matmul(
                                        pt[:, half * 256 : (half + 1) * 256],
                                        lhsT=xt[:, s : s + 128],
                                        rhs=m16[:, ch, di, :],
                                        start=(di == 0 and ch == 0),
                                        stop=(di == 7 and ch == 1),
                                    )
                        pw = len(pair) * 256
                        nc.vector.tensor_add(
                            ot[:, jp * 512 : jp * 512 + pw],
                            pt[:, :pw],
                            bias_sb[:, :pw],
                        )
                    # store: out rows = img*NQ + 128*j + p, contiguous per tile
                    base = img * NQ + 128 * gtiles[0]
                    dst = out_d[:][base : base + 128 * len(gtiles), :].rearrange(
                        "(j p) o -> p j o", p=128
                    )
                    src = ot[:, :ow].rearrange("p (j o) -> p j o", o=256)
                    nc.sync.dma_start(out=dst, in_=src)

    nc.finalize()  # Bacc: run reg-alloc + sync-wait splitting before serialization
    return nc


def _host_prep(grid_embedding, Wd, Wc, bc):
    g = np.asarray(grid_embedding, dtype=np.float32)
    gpad = np.zeros((B, C, XF), np.float32)
    gview = gpad[:, :, 1 : 1 + HP * HP].reshape(B, C, HP, HP)
    gview[:, :, 1 : H + 1, 1 : W + 1] = g.transpose(0, 3, 1, 2)
    # [d, e, c] / [d, e, o], then permuted to the SBUF layout [p=e%128, d, ec, c|o]
    wdt_dec = np.asarray(Wd, np.float32).transpose(0, 2, 1)
    wct_dec = np.asarray(Wc, np.float32).reshape(C, 8, C).transpose(1, 2, 0)
    wdt = np.ascontiguousarray(
        wdt_dec.reshape(8, 2, 128, C).transpose(2, 0, 1, 3).astype(np.float16)
    )  # [128, 8, 2, C] fp16 (same rounding the device cast-DMA applied; halves
    # the critical-path weight read)
    wct = np.ascontiguousarray(
        wct_dec.reshape(8, 2, 128, C).transpose(2, 0, 1, 3).astype(np.float16)
    )  # [128, 8, 2, C] fp16
    bias = np.zeros((1, 512), np.float32)
    bias[0, :128] = 1.0
    bias[0, 256:512] = np.asarray(bc, np.float32)
    return gpad, wdt, wct, bias


def _unpad_out(outpad_flat):
    # [NQ*images, 256] -> [images, H, W, C]: rows are (hp-1, wp) for padded
    # rows hp in 1..64 and all wp in 0..66; discard wp 0 and 65.
    n_img = outpad_flat.shape[0] // NQ
    o = outpad_flat.reshape(n_img, H, HP, C)
    return o[:, :, 1 : W + 1, :]


_NC_CACHE = {}


def kernel(grid_embedding, Wd, Wc, bc):
    global LAST_RESULTS
    gpad, wdt, wct, bias = _host_prep(grid_embedding, Wd, Wc, bc)

    if "nc" not in _NC_CACHE:
        _NC_CACHE["nc"] = build_bass()
    nc = _NC_CACHE["nc"]

    in_maps = [
        {
            "xp": np.ascontiguousarray(gpad[core * BPC : (core + 1) * BPC]),
            "wdt": wdt,
            "wct": wct,
            "bias": bias,
        }
        for core in range(N_CORES)
    ]
    res = run_bass_kernel_spmd(nc, in_maps, core_ids=list(range(N_CORES)))
    LAST_RESULTS = res
    out = np.concatenate([_unpad_out(r["out"]) for r in res.results], axis=0)
    return np.ascontiguousarray(out.reshape(B, H, W, C))


if __name__ == "__main__":
    rng = np.random.default_rng(0)
    inputs = {
        "grid_embedding": rng.standard_normal((B, H, W, C), dtype=np.float32),
        "Wd": (rng.standard_normal((8, C, C)) * 0.01).astype(np.float32),
        "Wc": (rng.standard_normal((C, 8 * C)) * 0.02).astype(np.float32),
        "bc": (rng.standard_normal(C) * 0.02).astype(np.float32),
    }
    out = kernel(**inputs)
    print("out", out.shape, out.dtype)
